# revision 42
# baseline (speedup 1.0000x reference)
"""Tensor-parallel causal self-attention (GQA + RoPE) for 8 TRN2 NeuronCores.

Sharding: batch(2) x kv-head-groups(4). Core c handles batch c//4 and kv heads
{2*(c%4), 2*(c%4)+1} (= 8 q heads). Each core computes a partial output
y_c[t, e] (its heads' contribution through wo); host sums the 4 partials per
batch.

Hardcoded problem shape: B=2, S=2048, D=2048, H=32, KV=8, HD=64.

v3 design (vs v2 at 425us): phase 2 is ACT-bound (exp total ~158us on the
scalar engine vs ~116us of PE attention work). The out-projection of token
block qt-1 is interleaved into qt's kc loops as PE filler so PE and ACT
overlap instead of alternating starvation; poA/poB evacuate to SBUF on
gpsimd immediately (po pool 2 bufs, rr shares the yo pool, yo 2 bufs);
ys/oTu/wo/ind2b/y in bf16 (FWL on out-proj stationaries, half the wo/y
DMA); phase-1 psum evacuations all on ACT so tt boundaries don't queue
behind DVE rope work.

v2 design (vs v1 baseline at 591us):
- bf16 operands for QKV projections, scores and PV matmuls (rel-l2 ~6e-3,
  well under the 2e-2 gate); psum accumulation stays fp32. bf16 stationaries
  get compiler-auto fast-weight-load, raising PE array duty (HAM warmth).
- RoPE is pipelined per-512-token chunk inside phase 1 (was a 45us PE gap):
  deinterleave order per head is [x0 0-15 | x1 0-15 | x0 16-31 | x1 16-31] so
  the rotate-half swap is a single DVE stream_shuffle (mask i^16) instead of
  4 cross-block copies. KTsh (half-swapped K) is produced by a second add.
- Causal restriction: score/exp/PV APs are column-restricted near the
  diagonal (saves ~15% PE cycles and ACT exp elems); only the 128-wide
  boundary blocks get an affine_select triangle mask on GpSimd.
- Softmax sums ride in column 64 of the V stationary (as v1); reciprocal
  runs directly on the psum row (custom-DVE recip), replication via a tiny
  [2,128] indicator matmul per (hp, qt), normalization fused into the
  po->oTu copy. All overlapped with the next head's attention.
- Weight DMAs are chunked so the first projection matmul starts ~1us in.
"""

import numpy as np
import ml_dtypes

DIM = 2048
NH = 32
NKV = 8
HD = 64
SEQ = 2048
B = 2
NCORES = 8

BF = ml_dtypes.bfloat16

_CACHE = {}

# per-head row order: [x0 dims 0-15 | x1 dims 0-15 | x0 dims 16-31 | x1 16-31]
_PERM64 = (
    [2 * j for j in range(16)]
    + [2 * j + 1 for j in range(16)]
    + [2 * j for j in range(16, 32)]
    + [2 * j + 1 for j in range(16, 32)]
)


def _deinterleave(w, nheads):
    w = w.reshape(nheads, 64, DIM)
    return w[:, _PERM64, :]  # [nh, 64, D]


def _host_prep(x, freqs_cos, freqs_sin, wq, wk, wv, wo):
    """Build the per-core DRAM input dicts."""
    x = np.asarray(x, np.float32)
    freqs_cos = np.asarray(freqs_cos, np.float32)
    freqs_sin = np.asarray(freqs_sin, np.float32)
    wq = np.asarray(wq, np.float32)
    wk = np.asarray(wk, np.float32)
    wv = np.asarray(wv, np.float32)
    wo = np.asarray(wo, np.float32)

    wq_d = _deinterleave(wq, NH)           # [32, 64, D]
    wk_d = _deinterleave(wk, NKV)          # [8, 64, D]
    wv_r = wv.reshape(NKV, HD, DIM)        # [8, 64, D] (not permuted)

    # rope tables [128, 2048] fp32, matching the permuted row order
    ct = freqs_cos.T                       # [32, S]
    st = freqs_sin.T
    cos64 = np.concatenate([ct[0:16], ct[0:16], ct[16:32], ct[16:32]], axis=0)
    sin64 = np.concatenate([-st[0:16], st[0:16], -st[16:32], st[16:32]], axis=0)
    cosT = np.tile(cos64, (2, 1)).astype(BF)   # [128, S]
    sinT = np.tile(sin64, (2, 1)).astype(BF)

    ident = np.concatenate([np.eye(HD), np.eye(HD)], axis=0).astype(BF)  # [128, 64]
    ind2b = np.zeros((33, 128), BF)
    ind2b[0, 0:64] = 1.0
    ind2b[32, 64:128] = 1.0

    xT_b = []
    for b in range(B):
        xtb = np.ascontiguousarray(x[b].T)                      # [D, S]
        # quad-packed for 4KB DMA lines: [tt, quad, p, dc-in-quad, tok]
        xt = xtb.reshape(4, 4, 128, 4, 512).transpose(3, 0, 2, 1, 4)
        xT_b.append(np.ascontiguousarray(xt).astype(BF))

    in_maps = []
    for c in range(NCORES):
        b, g = c // 4, c % 4
        wq_c = wq_d[8 * g:8 * g + 8].reshape(512, DIM)           # [512, D]
        wq_t = np.ascontiguousarray(
            wq_c.T.reshape(16, 128, 512).transpose(1, 0, 2)
        ).astype(BF)                                             # [128p, 16dc, 512h]
        wkv_c = np.concatenate(
            [wk_d[2 * g:2 * g + 2].reshape(128, DIM),
             wv_r[2 * g:2 * g + 2].reshape(128, DIM)], axis=0)   # [256, D]
        wkv_t = np.ascontiguousarray(
            wkv_c.T.reshape(16, 128, 256).transpose(1, 0, 2)
        ).astype(BF)                                             # [128p, 16dc, 256]
        woc = np.ascontiguousarray(wo[:, 512 * g:512 * g + 512].T)  # [512a, 2048e]
        wo_t = np.ascontiguousarray(
            woc.reshape(4, 128, 2048).transpose(1, 0, 2)).astype(BF)  # [128p, 4hc, 2048e]
        in_maps.append({
            "xT": xT_b[b],
            "wq": wq_t,
            "wkv": wkv_t,
            "wo": wo_t,
            "cosT": cosT,
            "sinT": sinT,
            "ident": ident,
            "ind2b": ind2b,
            "ones16": np.ones((128, 16), BF),
        })
    return in_maps


def _build_kernel(tc, nc, io, mybir):
    from contextlib import ExitStack

    fp = mybir.dt.float32
    fpr = mybir.dt.float32r
    bf = mybir.dt.bfloat16
    Exp = mybir.ActivationFunctionType.Exp
    is_ge = mybir.AluOpType.is_ge
    SWAP16 = [i ^ 16 for i in range(32)]

    with ExitStack() as ctx:
        consts = ctx.enter_context(tc.tile_pool(name="consts", bufs=1))
        big = ctx.enter_context(tc.tile_pool(name="big", bufs=1))

        QT = big.tile([128, 4, 2048], bf)
        KT = big.tile([128, 2048], bf)
        KTsh = big.tile([128, 2048], bf)
        Vp = big.tile([128, 2, 16, 65], bf)
        oTu = big.tile([128, 4, 2048], bf)
        wo_s = big.tile([128, 4, 2048], bf)

        xT = io["xT"].ap()
        yap = io["y"].ap()

        # ---- phase 1: Q/K/V projections + pipelined rope / V transpose ----
        with tc.tile_pool(name="xc", bufs=10) as xcp, \
             tc.tile_pool(name="vts", bufs=2) as vsp, \
             tc.tile_pool(name="rope", bufs=2) as rp, \
             tc.tile_pool(name="pj", bufs=1, space="PSUM") as pjp, \
             tc.tile_pool(name="vt", bufs=2, space="PSUM") as vtp:
            # DMA is line-size bound: 1KB/partition lines drain at ~71GB/s
            # per queue, 4KB at ~140+. x is quad-packed (4KB lines),
            # weights are [128p, dc, out] (big lines, quartered for startup
            # granularity), and transfers alternate between the sync and
            # scalar HWDGE rings in consumption order.
            wq_s = big.tile([128, 16, 512], bf)
            wkv_s = big.tile([128, 16, 256], bf)

            def xdma(tt, q, eng=None):
                xc = xcp.tile([128, 4, 512], bf, name="xc", tag="xc")
                (eng or nc.sync).dma_start(xc[:], xT[tt, q])
                return xc

            # tt0 setup burst: x + weights JIT-interleaved over both rings.
            # After this, ONLY sync issues DMAs: a descriptor issue on ACT
            # blocks when its ring is full, and that stalls the psum
            # evacuations queued behind it on the ACT engine.
            xcs0 = [None] * 4
            xcs0[0] = xdma(0, 0)
            nc.scalar.dma_start(wq_s[:, 0:4, :], io["wq"].ap()[:, 0:4])
            nc.scalar.dma_start(wkv_s[:, 0:4, :], io["wkv"].ap()[:, 0:4])
            xcs0[1] = xdma(0, 1)
            nc.scalar.dma_start(wq_s[:, 4:8, :], io["wq"].ap()[:, 4:8])
            nc.sync.dma_start(wq_s[:, 8:12, :], io["wq"].ap()[:, 8:12])
            nc.scalar.dma_start(wkv_s[:, 4:8, :], io["wkv"].ap()[:, 4:8])
            xcs0[2] = xdma(0, 2)
            nc.scalar.dma_start(wkv_s[:, 8:12, :], io["wkv"].ap()[:, 8:12])
            nc.scalar.dma_start(wq_s[:, 12:16, :], io["wq"].ap()[:, 12:16])
            xcs0[3] = xdma(0, 3)
            nc.scalar.dma_start(wkv_s[:, 12:16, :], io["wkv"].ap()[:, 12:16])
            id_s = consts.tile([128, 64], bf)
            nc.scalar.dma_start(id_s[:], io["ident"].ap())
            ind2b_r = consts.tile([33, 128], bf)
            nc.scalar.dma_start(ind2b_r[:], io["ind2b"].ap())
            for kv in range(2):
                nc.scalar.dma_start(Vp[:, kv, :, 64], io["ones16"].ap())
            # rope tables on sync (consumed from ~35us; sync has slack)
            cos_s = consts.tile([128, 2048], bf)
            sin_s = consts.tile([128, 2048], bf)

            def rope_math(c0, dst, is_k, ts, last_tt=False):
                # for the last tt, keep DVE clear: its rope adds otherwise
                # collide with qt0's softmax chains right after the phase
                # transition
                add_eng = nc.gpsimd if last_tt else nc.vector
                sw = rp.tile([128, 512], fp, name="sw", tag="sw")
                nc.vector.stream_shuffle(sw[:], c0[:], SWAP16)
                t1 = rp.tile([128, 512], fp, name="t1", tag="t1")
                nc.gpsimd.tensor_mul(t1[:], c0[:], cos_s[:, ts])
                t2 = rp.tile([128, 512], fp, name="t2", tag="t2")
                nc.gpsimd.tensor_mul(t2[:], sw[:], sin_s[:, ts])
                add_eng.tensor_add(dst, t1[:], t2[:])
                if is_k:
                    keng = nc.gpsimd if last_tt else nc.vector
                    keng.tensor_copy(KTsh[0:64, ts], KT[64:128, ts])
                    keng.tensor_copy(KTsh[64:128, ts], KT[0:64, ts])

            for tt in range(4):
                acc = [pjp.tile([128, 512], fp, name=f"acc{i}", tag=f"acc{i}")
                       for i in range(6)]
                for dc in range(16):
                    if dc % 4 == 0:
                        xq = xcs0[dc // 4] if tt == 0 else xdma(tt, dc // 4)
                    xc = xq[:, dc % 4, :]
                    mk = dict(start=(dc == 0), stop=(dc == 15),
                              skip_group_check=True)
                    for hc in range(4):
                        nc.tensor.matmul(
                            acc[hc][:],
                            wq_s[:, dc, hc * 128:(hc + 1) * 128],
                            xc, **mk)
                    nc.tensor.matmul(acc[4][:], wkv_s[:, dc, 0:128],
                                     xc, **mk)
                    nc.tensor.matmul(acc[5][:], wkv_s[:, dc, 128:256],
                                     xc, **mk)
                if tt == 0:
                    nc.sync.dma_start(cos_s[:], io["cosT"].ap())
                    nc.sync.dma_start(sin_s[:], io["sinT"].ap())
                ts = slice(tt * 512, (tt + 1) * 512)
                # psum evacuation all on ACT (idle in phase 1) so next tt's
                # matmuls don't queue behind DVE rope work
                c0s = []
                for i in range(5):
                    c0 = rp.tile([128, 512], fp, name="c0", tag=f"c0{i}")
                    nc.scalar.copy(c0[:], acc[i][:])
                    c0s.append(c0)
                vtt = vsp.tile([128, 512], bf, name="vtt", tag="vtt")
                nc.scalar.copy(vtt[:], acc[5][:])
                rope_math(c0s[4], KT[:, ts], True, ts, last_tt=(tt == 3))
                for hp in range(4):
                    rope_math(c0s[hp], QT[:, hp, ts], False, ts,
                              last_tt=(tt == 3))
                for kv in range(2):
                    for j in range(4):
                        kc = 4 * tt + j
                        tp = vtp.tile([128, 64], bf)
                        nc.tensor.transpose(
                            tp[:], vtt[kv * 64:(kv + 1) * 64,
                                       j * 128:(j + 1) * 128],
                            id_s[kv * 64:(kv + 1) * 64, :])
                        nc.scalar.copy(Vp[:, kv, kc, 0:64], tp[:])
                if tt == 3:
                    for hc in range(4):
                        nc.sync.dma_start(wo_s[:, hc, :],
                                          io["wo"].ap()[:, hc])

        # ---- phase 2: attention (row-tiled head pairs), qt-outer. The
        # output projection of token block qt-1 is interleaved INTO qt's
        # kc loops as PE filler while ACT chews exps (ACT is the phase-2
        # bottleneck engine: exp total ~157us vs PE attention ~116us). ----
        with tc.tile_pool(name="st", bufs=2, space="PSUM") as stp, \
             tc.tile_pool(name="po", bufs=2, space="PSUM") as pop, \
             tc.tile_pool(name="yo", bufs=2, space="PSUM") as yop, \
             tc.tile_pool(name="pt", bufs=4) as ptp, \
             tc.tile_pool(name="ys", bufs=6) as ysp, \
             tc.tile_pool(name="sm", bufs=2) as smp:

            def outproj_group(tcn, et):
                yo = yop.tile([128, 512], fp, name="yo", tag="yo")
                for hc in range(4):
                    nc.tensor.matmul(
                        yo[:], oTu[:, hc, tcn * 128:(tcn + 1) * 128],
                        wo_s[:, hc, et * 512:(et + 1) * 512],
                        start=(hc == 0), stop=(hc == 3),
                        skip_group_check=True)
                ys = ysp.tile([128, 512], bf)
                nc.vector.tensor_copy(ys[:], yo[:])
                nc.sync.dma_start(yap[tcn, et], ys[:])

            # Software-pipelined attention: the (scores, exp) stream runs LA
            # iterations ahead of the (PV, fills, softmax-chain) stream.
            # Without this, the last exp of each hp gates its last PV, which
            # gates (in-order PE) the next hp's first scores, which gate the
            # next exp -- a 2-5us ACT bubble at every hp boundary. The rr
            # broadcast matmul of hp is likewise deferred into hp+1's PV
            # stream so it never blocks the PE queue on the DVE chain.
            LA = 2
            iters = []
            for qt in range(4):
                for hp in range(4):
                    for kc in range(4 * (qt + 1)):
                        iters.append((qt, hp, kc))

            # out-proj groups of qt become "ready" when qt's last hp
            # completes; spread them over later hps proportional to each
            # qt's ACT-minus-PE slack (qt1 can absorb ~15 groups, qt2 ~24,
            # qt3 ~32; the tail takes the rest)
            ready_groups = []
            FILL_CAP = [0, 0, 0, 0,  2, 2, 2, 2,  4, 4, 4, 4,  6, 6, 6, 6]

            pending_rr = [None]

            def flush_rr():
                if pending_rr[0] is not None:
                    pending_rr[0]()
                    pending_rr[0] = None

            pts = {}      # idx -> (pt tile, o)
            hpstate = {}  # (qt, hp) -> dict

            def emit_sea(idx):
                qt, hp, kc = iters[idx]
                q0 = qt * 512
                nck = 4 * (qt + 1)
                kv = hp // 2
                KA = KT if kv == 0 else KTsh      # head 2hp   rows 0:64
                KB = KTsh if kv == 0 else KT      # head 2hp+1 rows 64:128
                o = max(0, 128 * kc - q0)
                st = stp.tile([128, 2, 512], fp, name="st", tag="st")
                nc.tensor.matmul(
                    st[:, 0, o:512],
                    KA[0:64, kc * 128:(kc + 1) * 128],
                    QT[0:64, hp, q0 + o:q0 + 512],
                    start=True, stop=True, skip_group_check=True)
                nc.tensor.matmul(
                    st[:, 1, o:512],
                    KB[64:128, kc * 128:(kc + 1) * 128],
                    QT[64:128, hp, q0 + o:q0 + 512],
                    start=True, stop=True, skip_group_check=True)
                pt = ptp.tile([128, 2, 512], bf, name="pt", tag="pt")
                nc.scalar.activation(pt[:, :, o:512], st[:, :, o:512],
                                     Exp, scale=0.125)
                if 128 * kc >= q0:
                    for j in range(2):
                        blk = pt[:, j, o:o + 128]
                        nc.gpsimd.affine_select(
                            out=blk, in_=blk, base=0,
                            channel_multiplier=-1, pattern=[[1, 128]],
                            compare_op=is_ge, fill=0.0)
                pts[idx] = (pt, o)

            def emit_pv(idx):
                qt, hp, kc = iters[idx]
                q0 = qt * 512
                qs = slice(q0, q0 + 512)
                nck = 4 * (qt + 1)
                kv = hp // 2
                if kc == 0:
                    if hp == 0 and qt >= 1:
                        ready_groups.extend(
                            [(4 * (qt - 1) + h, et)
                             for h in range(4) for et in range(4)])
                    nfill = min(FILL_CAP[qt * 4 + hp], len(ready_groups))
                    # fill positions spread evenly mid-hp: never in the last
                    # 2 kcs (a fill there delays the next hp's first scores
                    # and starves ACT), never back-to-back (a fill group
                    # locally overruns the ACT pace), and for hp==0 not
                    # before the kc==2 rr flush (fills read oTu written by
                    # that mul)
                    fstart = 3 if hp == 0 else 1
                    fend = nck - 4
                    if nfill <= 1:
                        pos = [fstart] if nfill else []
                    else:
                        pos = sorted({fstart + round(k * (fend - fstart)
                                                     / (nfill - 1))
                                      for k in range(nfill)})
                    hpstate[(qt, hp)] = dict(
                        poA=pop.tile([65, 512], fp, name="poA", tag="po"),
                        poB=pop.tile([65, 512], fp, name="poB", tag="po"),
                        fills=[ready_groups.pop(0) for _ in range(nfill)],
                        pos=pos)
                stt = hpstate[(qt, hp)]
                poA, poB = stt["poA"], stt["poB"]
                pt, o = pts.pop(idx)
                mk = dict(start=(kc == 0), stop=(kc == nck - 1),
                          skip_group_check=True)
                nc.tensor.matmul(poA[:, o:512], Vp[:, kv, kc, :],
                                 pt[:, 0, o:512], **mk)
                nc.tensor.matmul(poB[:, o:512], Vp[:, kv, kc, :],
                                 pt[:, 1, o:512], **mk)
                if kc == 2:
                    flush_rr()
                fills = stt["fills"]
                while stt["pos"] and kc == stt["pos"][0]:
                    stt["pos"].pop(0)
                    if fills:
                        outproj_group(*fills.pop(0))
                if kc != nck - 1:
                    return
                last_hp = (qt == 3 and hp == 3)
                s2 = smp.tile([33, 512], fp, name="s2", tag="s2")
                r2 = smp.tile([33, 512], fp, name="r2", tag="r2")
                r2b = smp.tile([33, 512], bf, name="r2b", tag="r2b")
                sAB = smp.tile([128, 512], fp, name="sAB", tag="sAB")

                def evac():
                    nc.vector.tensor_copy(sAB[0:64, :], poA[0:64, :])
                    nc.vector.tensor_copy(sAB[64:128, :], poB[0:64, :])

                def sums():
                    nc.vector.tensor_copy(s2[0:1, :], poA[64:65, :])
                    nc.vector.tensor_copy(s2[32:33, :], poB[64:65, :])
                    nc.vector.reciprocal_approx_fast(r2[:], s2[:])
                    if qt == 0:
                        nc.gpsimd.tensor_copy(r2b[:], r2[:])
                    else:
                        nc.vector.tensor_copy(r2b[:], r2[:])

                # leftover fills (dedup of positions) drain here; on the
                # last hp the rr chain is the critical path into the tail,
                # so its DVE ops go first
                if last_hp:
                    sums()
                    for tcn_et in fills:
                        outproj_group(*tcn_et)
                    evac()
                else:
                    for tcn_et in fills:
                        outproj_group(*tcn_et)
                    evac()
                    sums()

                def emit_rr(r2b=r2b, sAB=sAB, hp=hp, qs=qs):
                    rr = yop.tile([128, 512], fp, name="rr", tag="yo")
                    nc.tensor.matmul(rr[:], ind2b_r[:], r2b[:],
                                     start=True, stop=True,
                                     skip_group_check=True)
                    nc.vector.tensor_mul(oTu[:, hp, qs], sAB[:], rr[:])

                pending_rr[0] = emit_rr

            for idx in range(len(iters) + LA):
                if idx < len(iters):
                    emit_sea(idx)
                if idx >= LA:
                    emit_pv(idx - LA)
            # tail: output projection for the last token block (qt=3)
            flush_rr()
            for tcn in range(12, 16):
                for et in range(4):
                    outproj_group(tcn, et)


def _get_program():
    if "nc" in _CACHE:
        return _CACHE["nc"]
    import concourse.tile as tile
    from concourse import bacc, mybir

    nc = bacc.Bacc("TRN2", target_bir_lowering=False, debug=False,
                   num_devices=NCORES)
    fp = mybir.dt.float32
    bf = mybir.dt.bfloat16
    io = {
        "xT": nc.dram_tensor("xT", [4, 4, 128, 4, 512], bf,
                             kind="ExternalInput"),
        "wq": nc.dram_tensor("wq", [128, 16, 512], bf, kind="ExternalInput"),
        "wkv": nc.dram_tensor("wkv", [128, 16, 256], bf, kind="ExternalInput"),
        "wo": nc.dram_tensor("wo", [128, 4, 2048], bf, kind="ExternalInput"),
        "cosT": nc.dram_tensor("cosT", [128, 2048], bf, kind="ExternalInput"),
        "sinT": nc.dram_tensor("sinT", [128, 2048], bf, kind="ExternalInput"),
        "ident": nc.dram_tensor("ident", [128, 64], bf, kind="ExternalInput"),
        "ind2b": nc.dram_tensor("ind2b", [33, 128], bf, kind="ExternalInput"),
        "ones16": nc.dram_tensor("ones16", [128, 16], bf, kind="ExternalInput"),
        "y": nc.dram_tensor("y", [16, 4, 128, 512], bf, kind="ExternalOutput"),
    }
    with tile.TileContext(nc) as tc:
        _build_kernel(tc, nc, io, mybir)
    nc.compile()
    _CACHE["nc"] = nc
    return nc


def _run(inputs, trace=False):
    from concourse.bass_utils import run_bass_kernel_spmd

    nc = _get_program()
    in_maps = _host_prep(**inputs)
    res = run_bass_kernel_spmd(nc, in_maps, core_ids=list(range(NCORES)),
                               trace=trace)
    parts = [r_["y"].astype(np.float32).transpose(0, 2, 1, 3).reshape(SEQ, DIM)
             for r_ in res.results]
    out = np.stack([
        parts[0] + parts[1] + parts[2] + parts[3],
        parts[4] + parts[5] + parts[6] + parts[7],
    ]).astype(np.float32)
    return out, res


def kernel(**inputs):
    out, _ = _run(inputs, trace=False)
    return out



# revision 44
# speedup vs baseline: 1.0348x; 1.0348x over previous
"""Tensor-parallel causal self-attention (GQA + RoPE) for 8 TRN2 NeuronCores.

Sharding: batch(2) x kv-head-groups(4). Core c handles batch c//4 and kv heads
{2*(c%4), 2*(c%4)+1} (= 8 q heads). Each core computes a partial output
y_c[t, e] (its heads' contribution through wo); host sums the 4 partials per
batch.

Hardcoded problem shape: B=2, S=2048, D=2048, H=32, KV=8, HD=64.

v3 design (vs v2 at 425us): phase 2 is ACT-bound (exp total ~158us on the
scalar engine vs ~116us of PE attention work). The out-projection of token
block qt-1 is interleaved into qt's kc loops as PE filler so PE and ACT
overlap instead of alternating starvation; poA/poB evacuate to SBUF on
gpsimd immediately (po pool 2 bufs, rr shares the yo pool, yo 2 bufs);
ys/oTu/wo/ind2b/y in bf16 (FWL on out-proj stationaries, half the wo/y
DMA); phase-1 psum evacuations all on ACT so tt boundaries don't queue
behind DVE rope work.

v2 design (vs v1 baseline at 591us):
- bf16 operands for QKV projections, scores and PV matmuls (rel-l2 ~6e-3,
  well under the 2e-2 gate); psum accumulation stays fp32. bf16 stationaries
  get compiler-auto fast-weight-load, raising PE array duty (HAM warmth).
- RoPE is pipelined per-512-token chunk inside phase 1 (was a 45us PE gap):
  deinterleave order per head is [x0 0-15 | x1 0-15 | x0 16-31 | x1 16-31] so
  the rotate-half swap is a single DVE stream_shuffle (mask i^16) instead of
  4 cross-block copies. KTsh (half-swapped K) is produced by a second add.
- Causal restriction: score/exp/PV APs are column-restricted near the
  diagonal (saves ~15% PE cycles and ACT exp elems); only the 128-wide
  boundary blocks get an affine_select triangle mask on GpSimd.
- Softmax sums ride in column 64 of the V stationary (as v1); reciprocal
  runs directly on the psum row (custom-DVE recip), replication via a tiny
  [2,128] indicator matmul per (hp, qt), normalization fused into the
  po->oTu copy. All overlapped with the next head's attention.
- Weight DMAs are chunked so the first projection matmul starts ~1us in.
"""

import numpy as np
import ml_dtypes

DIM = 2048
NH = 32
NKV = 8
HD = 64
SEQ = 2048
B = 2
NCORES = 8

BF = ml_dtypes.bfloat16

_CACHE = {}

# per-head row order: [x0 dims 0-15 | x1 dims 0-15 | x0 dims 16-31 | x1 16-31]
_PERM64 = (
    [2 * j for j in range(16)]
    + [2 * j + 1 for j in range(16)]
    + [2 * j for j in range(16, 32)]
    + [2 * j + 1 for j in range(16, 32)]
)


def _deinterleave(w, nheads):
    w = w.reshape(nheads, 64, DIM)
    return w[:, _PERM64, :]  # [nh, 64, D]


def _host_prep(x, freqs_cos, freqs_sin, wq, wk, wv, wo):
    """Build the per-core DRAM input dicts."""
    x = np.asarray(x, np.float32)
    freqs_cos = np.asarray(freqs_cos, np.float32)
    freqs_sin = np.asarray(freqs_sin, np.float32)
    wq = np.asarray(wq, np.float32)
    wk = np.asarray(wk, np.float32)
    wv = np.asarray(wv, np.float32)
    wo = np.asarray(wo, np.float32)

    wq_d = _deinterleave(wq, NH)           # [32, 64, D]
    wk_d = _deinterleave(wk, NKV)          # [8, 64, D]
    wv_r = wv.reshape(NKV, HD, DIM)        # [8, 64, D] (not permuted)

    # rope tables [128, 2048] fp32, matching the permuted row order
    ct = freqs_cos.T                       # [32, S]
    st = freqs_sin.T
    cos64 = np.concatenate([ct[0:16], ct[0:16], ct[16:32], ct[16:32]], axis=0)
    sin64 = np.concatenate([-st[0:16], st[0:16], -st[16:32], st[16:32]], axis=0)
    cosT = np.tile(cos64, (2, 1)).astype(BF)   # [128, S]
    sinT = np.tile(sin64, (2, 1)).astype(BF)

    ident = np.concatenate([np.eye(HD), np.eye(HD)], axis=0).astype(BF)  # [128, 64]
    ind2b = np.zeros((33, 128), BF)
    ind2b[0, 0:64] = 1.0
    ind2b[32, 64:128] = 1.0

    xT_b = []
    for b in range(B):
        xtb = np.ascontiguousarray(x[b].T)                      # [D, S]
        # quad-packed for 4KB DMA lines: [tt, quad, p, dc-in-quad, tok]
        xt = xtb.reshape(4, 4, 128, 4, 512).transpose(3, 0, 2, 1, 4)
        xT_b.append(np.ascontiguousarray(xt).astype(BF))

    in_maps = []
    for c in range(NCORES):
        b, g = c // 4, c % 4
        wq_c = wq_d[8 * g:8 * g + 8].reshape(512, DIM)           # [512, D]
        wq_t = np.ascontiguousarray(
            wq_c.T.reshape(16, 128, 512).transpose(1, 0, 2)
        ).astype(BF)                                             # [128p, 16dc, 512h]
        wkv_c = np.concatenate(
            [wk_d[2 * g:2 * g + 2].reshape(128, DIM),
             wv_r[2 * g:2 * g + 2].reshape(128, DIM)], axis=0)   # [256, D]
        wkv_t = np.ascontiguousarray(
            wkv_c.T.reshape(16, 128, 256).transpose(1, 0, 2)
        ).astype(BF)                                             # [128p, 16dc, 256]
        woc = np.ascontiguousarray(wo[:, 512 * g:512 * g + 512].T)  # [512a, 2048e]
        wo_t = np.ascontiguousarray(
            woc.reshape(4, 128, 2048).transpose(1, 0, 2)).astype(BF)  # [128p, 4hc, 2048e]
        in_maps.append({
            "xT": xT_b[b],
            "wq": wq_t,
            "wkv": wkv_t,
            "wo": wo_t,
            "cosT": cosT,
            "sinT": sinT,
            "ident": ident,
            "ind2b": ind2b,
            "ones16": np.ones((128, 16), BF),
        })
    return in_maps


def _build_kernel(tc, nc, io, mybir):
    from contextlib import ExitStack

    fp = mybir.dt.float32
    fpr = mybir.dt.float32r
    bf = mybir.dt.bfloat16
    Exp = mybir.ActivationFunctionType.Exp
    is_ge = mybir.AluOpType.is_ge
    SWAP16 = [i ^ 16 for i in range(32)]

    with ExitStack() as ctx:
        consts = ctx.enter_context(tc.tile_pool(name="consts", bufs=1))
        big = ctx.enter_context(tc.tile_pool(name="big", bufs=1))

        QT = big.tile([128, 4, 2048], bf)
        KT = big.tile([128, 2048], bf)
        KTsh = big.tile([128, 2048], bf)
        Vp = big.tile([128, 2, 16, 65], bf)
        oTu = big.tile([128, 4, 2048], bf)
        wo_s = big.tile([128, 4, 2048], bf)

        xT = io["xT"].ap()
        yap = io["y"].ap()

        # ---- phase 1: Q/K/V projections + pipelined rope / V transpose ----
        with tc.tile_pool(name="xc", bufs=10) as xcp, \
             tc.tile_pool(name="vts", bufs=2) as vsp, \
             tc.tile_pool(name="rope", bufs=2) as rp, \
             tc.tile_pool(name="pj", bufs=1, space="PSUM") as pjp, \
             tc.tile_pool(name="vt", bufs=2, space="PSUM") as vtp:
            # DMA is line-size bound: 1KB/partition lines drain at ~71GB/s
            # per queue, 4KB at ~140+. x is quad-packed (4KB lines),
            # weights are [128p, dc, out] (big lines, quartered for startup
            # granularity), and transfers alternate between the sync and
            # scalar HWDGE rings in consumption order.
            wq_s = big.tile([128, 16, 512], bf)
            wkv_s = big.tile([128, 16, 256], bf)

            def xdma(tt, q, eng=None):
                xc = xcp.tile([128, 4, 512], bf, name="xc", tag="xc")
                (eng or nc.sync).dma_start(xc[:], xT[tt, q])
                return xc

            # tt0 setup burst: x + weights JIT-interleaved over both rings.
            # After this, ONLY sync issues DMAs: a descriptor issue on ACT
            # blocks when its ring is full, and that stalls the psum
            # evacuations queued behind it on the ACT engine.
            xcs0 = [None] * 4
            xcs0[0] = xcp.tile([128, 4, 512], bf, name="xc", tag="xc")
            nc.sync.dma_start(xcs0[0][:, 0:2, :], xT[0, 0, :, 0:2, :])
            nc.scalar.dma_start(wq_s[:, 0:2, :], io["wq"].ap()[:, 0:2])
            nc.sync.dma_start(xcs0[0][:, 2:4, :], xT[0, 0, :, 2:4, :])
            nc.scalar.dma_start(wkv_s[:, 0:2, :], io["wkv"].ap()[:, 0:2])
            nc.scalar.dma_start(wq_s[:, 2:4, :], io["wq"].ap()[:, 2:4])
            nc.scalar.dma_start(wkv_s[:, 2:4, :], io["wkv"].ap()[:, 2:4])
            xcs0[1] = xdma(0, 1)
            nc.scalar.dma_start(wq_s[:, 4:8, :], io["wq"].ap()[:, 4:8])
            nc.sync.dma_start(wq_s[:, 8:12, :], io["wq"].ap()[:, 8:12])
            nc.scalar.dma_start(wkv_s[:, 4:8, :], io["wkv"].ap()[:, 4:8])
            xcs0[2] = xdma(0, 2)
            nc.scalar.dma_start(wkv_s[:, 8:12, :], io["wkv"].ap()[:, 8:12])
            nc.scalar.dma_start(wq_s[:, 12:16, :], io["wq"].ap()[:, 12:16])
            xcs0[3] = xdma(0, 3)
            nc.scalar.dma_start(wkv_s[:, 12:16, :], io["wkv"].ap()[:, 12:16])
            id_s = consts.tile([128, 64], bf)
            nc.scalar.dma_start(id_s[:], io["ident"].ap())
            ind2b_r = consts.tile([33, 128], bf)
            nc.scalar.dma_start(ind2b_r[:], io["ind2b"].ap())
            for kv in range(2):
                nc.scalar.dma_start(Vp[:, kv, :, 64], io["ones16"].ap())
            # rope tables on sync (consumed from ~35us; sync has slack)
            cos_s = consts.tile([128, 2048], bf)
            sin_s = consts.tile([128, 2048], bf)

            def rope_math(c0, dst, is_k, ts, last_tt=False):
                # for the last tt, keep DVE clear: its rope adds otherwise
                # collide with qt0's softmax chains right after the phase
                # transition
                add_eng = nc.gpsimd if last_tt else nc.vector
                sw = rp.tile([128, 512], fp, name="sw", tag="sw")
                nc.vector.stream_shuffle(sw[:], c0[:], SWAP16)
                t1 = rp.tile([128, 512], fp, name="t1", tag="t1")
                nc.gpsimd.tensor_mul(t1[:], c0[:], cos_s[:, ts])
                t2 = rp.tile([128, 512], fp, name="t2", tag="t2")
                nc.gpsimd.tensor_mul(t2[:], sw[:], sin_s[:, ts])
                add_eng.tensor_add(dst, t1[:], t2[:])
                if is_k:
                    keng = nc.gpsimd if last_tt else nc.vector
                    keng.tensor_copy(KTsh[0:64, ts], KT[64:128, ts])
                    keng.tensor_copy(KTsh[64:128, ts], KT[0:64, ts])

            for tt in range(4):
                acc = [pjp.tile([128, 512], fp, name=f"acc{i}", tag=f"acc{i}")
                       for i in range(6)]
                for dc in range(16):
                    if dc % 4 == 0:
                        xq = xcs0[dc // 4] if tt == 0 else xdma(tt, dc // 4)
                    xc = xq[:, dc % 4, :]
                    mk = dict(start=(dc == 0), stop=(dc == 15),
                              skip_group_check=True)
                    for hc in range(4):
                        nc.tensor.matmul(
                            acc[hc][:],
                            wq_s[:, dc, hc * 128:(hc + 1) * 128],
                            xc, **mk)
                    nc.tensor.matmul(acc[4][:], wkv_s[:, dc, 0:128],
                                     xc, **mk)
                    nc.tensor.matmul(acc[5][:], wkv_s[:, dc, 128:256],
                                     xc, **mk)
                if tt == 0:
                    nc.sync.dma_start(cos_s[:], io["cosT"].ap())
                    nc.sync.dma_start(sin_s[:], io["sinT"].ap())
                ts = slice(tt * 512, (tt + 1) * 512)
                # psum evacuation all on ACT (idle in phase 1) so next tt's
                # matmuls don't queue behind DVE rope work
                c0s = []
                for i in range(5):
                    c0 = rp.tile([128, 512], fp, name="c0", tag=f"c0{i}")
                    nc.scalar.copy(c0[:], acc[i][:])
                    c0s.append(c0)
                vtt = vsp.tile([128, 512], bf, name="vtt", tag="vtt")
                nc.scalar.copy(vtt[:], acc[5][:])
                rope_math(c0s[4], KT[:, ts], True, ts)
                for hp in range(4):
                    rope_math(c0s[hp], QT[:, hp, ts], False, ts)
                for kv in range(2):
                    for j in range(4):
                        kc = 4 * tt + j
                        tp = vtp.tile([128, 64], bf)
                        nc.tensor.transpose(
                            tp[:], vtt[kv * 64:(kv + 1) * 64,
                                       j * 128:(j + 1) * 128],
                            id_s[kv * 64:(kv + 1) * 64, :])
                        nc.scalar.copy(Vp[:, kv, kc, 0:64], tp[:])
                if tt == 3:
                    for hc in range(4):
                        nc.sync.dma_start(wo_s[:, hc, :],
                                          io["wo"].ap()[:, hc])

        # ---- phase 2: attention (row-tiled head pairs), qt-outer. The
        # output projection of token block qt-1 is interleaved INTO qt's
        # kc loops as PE filler while ACT chews exps (ACT is the phase-2
        # bottleneck engine: exp total ~157us vs PE attention ~116us). ----
        with tc.tile_pool(name="st", bufs=2, space="PSUM") as stp, \
             tc.tile_pool(name="po", bufs=2, space="PSUM") as pop, \
             tc.tile_pool(name="yo", bufs=2, space="PSUM") as yop, \
             tc.tile_pool(name="pt", bufs=4) as ptp, \
             tc.tile_pool(name="ys", bufs=6) as ysp, \
             tc.tile_pool(name="sm", bufs=2) as smp:

            def outproj_group(tcn, et):
                yo = yop.tile([128, 512], fp, name="yo", tag="yo")
                for hc in range(4):
                    nc.tensor.matmul(
                        yo[:], oTu[:, hc, tcn * 128:(tcn + 1) * 128],
                        wo_s[:, hc, et * 512:(et + 1) * 512],
                        start=(hc == 0), stop=(hc == 3),
                        skip_group_check=True)
                ys = ysp.tile([128, 512], bf)
                nc.vector.tensor_copy(ys[:], yo[:])
                nc.sync.dma_start(yap[tcn, et], ys[:])

            # Software-pipelined attention: the (scores, exp) stream runs LA
            # iterations ahead of the (PV, fills, softmax-chain) stream.
            # Without this, the last exp of each hp gates its last PV, which
            # gates (in-order PE) the next hp's first scores, which gate the
            # next exp -- a 2-5us ACT bubble at every hp boundary. The rr
            # broadcast matmul of hp is likewise deferred into hp+1's PV
            # stream so it never blocks the PE queue on the DVE chain.
            LA = 2
            iters = []
            for qt in range(4):
                for hp in range(4):
                    for kc in range(4 * (qt + 1)):
                        iters.append((qt, hp, kc))

            # out-proj groups of qt become "ready" when qt's last hp
            # completes; spread them over later hps proportional to each
            # qt's ACT-minus-PE slack (qt1 can absorb ~15 groups, qt2 ~24,
            # qt3 ~32; the tail takes the rest)
            ready_groups = []
            FILL_CAP = [0, 0, 0, 0,  2, 2, 2, 2,  4, 4, 4, 4,  6, 6, 6, 6]

            pending_rr = [None]

            def flush_rr():
                if pending_rr[0] is not None:
                    pending_rr[0]()
                    pending_rr[0] = None

            pts = {}      # idx -> (pt tile, o)
            hpstate = {}  # (qt, hp) -> dict

            def emit_sea(idx):
                qt, hp, kc = iters[idx]
                q0 = qt * 512
                nck = 4 * (qt + 1)
                kv = hp // 2
                KA = KT if kv == 0 else KTsh      # head 2hp   rows 0:64
                KB = KTsh if kv == 0 else KT      # head 2hp+1 rows 64:128
                o = max(0, 128 * kc - q0)
                st = stp.tile([128, 2, 512], fp, name="st", tag="st")
                nc.tensor.matmul(
                    st[:, 0, o:512],
                    KA[0:64, kc * 128:(kc + 1) * 128],
                    QT[0:64, hp, q0 + o:q0 + 512],
                    start=True, stop=True, skip_group_check=True)
                nc.tensor.matmul(
                    st[:, 1, o:512],
                    KB[64:128, kc * 128:(kc + 1) * 128],
                    QT[64:128, hp, q0 + o:q0 + 512],
                    start=True, stop=True, skip_group_check=True)
                pt = ptp.tile([128, 2, 512], bf, name="pt", tag="pt")
                nc.scalar.activation(pt[:, :, o:512], st[:, :, o:512],
                                     Exp, scale=0.125)
                if 128 * kc >= q0:
                    for j in range(2):
                        blk = pt[:, j, o:o + 128]
                        nc.gpsimd.affine_select(
                            out=blk, in_=blk, base=0,
                            channel_multiplier=-1, pattern=[[1, 128]],
                            compare_op=is_ge, fill=0.0)
                pts[idx] = (pt, o)

            def emit_pv(idx):
                qt, hp, kc = iters[idx]
                q0 = qt * 512
                qs = slice(q0, q0 + 512)
                nck = 4 * (qt + 1)
                kv = hp // 2
                if kc == 0:
                    if hp == 0 and qt >= 1:
                        ready_groups.extend(
                            [(4 * (qt - 1) + h, et)
                             for h in range(4) for et in range(4)])
                    nfill = min(FILL_CAP[qt * 4 + hp], len(ready_groups))
                    # fill positions spread evenly mid-hp: never in the last
                    # 2 kcs (a fill there delays the next hp's first scores
                    # and starves ACT), never back-to-back (a fill group
                    # locally overruns the ACT pace), and for hp==0 not
                    # before the kc==2 rr flush (fills read oTu written by
                    # that mul)
                    fstart = 3 if hp == 0 else 1
                    fend = nck - 4
                    if nfill <= 1:
                        pos = [fstart] if nfill else []
                    else:
                        pos = sorted({fstart + round(k * (fend - fstart)
                                                     / (nfill - 1))
                                      for k in range(nfill)})
                    hpstate[(qt, hp)] = dict(
                        poA=pop.tile([65, 512], fp, name="poA", tag="po"),
                        poB=pop.tile([65, 512], fp, name="poB", tag="po"),
                        fills=[ready_groups.pop(0) for _ in range(nfill)],
                        pos=pos)
                stt = hpstate[(qt, hp)]
                poA, poB = stt["poA"], stt["poB"]
                pt, o = pts.pop(idx)
                mk = dict(start=(kc == 0), stop=(kc == nck - 1),
                          skip_group_check=True)
                nc.tensor.matmul(poA[:, o:512], Vp[:, kv, kc, :],
                                 pt[:, 0, o:512], **mk)
                nc.tensor.matmul(poB[:, o:512], Vp[:, kv, kc, :],
                                 pt[:, 1, o:512], **mk)
                if kc == 2:
                    flush_rr()
                fills = stt["fills"]
                while stt["pos"] and kc == stt["pos"][0]:
                    stt["pos"].pop(0)
                    if fills:
                        outproj_group(*fills.pop(0))
                if kc != nck - 1:
                    return
                last_hp = (qt == 3 and hp == 3)
                s2 = smp.tile([33, 512], fp, name="s2", tag="s2")
                r2 = smp.tile([33, 512], fp, name="r2", tag="r2")
                r2b = smp.tile([33, 512], bf, name="r2b", tag="r2b")
                sAB = smp.tile([128, 512], fp, name="sAB", tag="sAB")

                def evac():
                    nc.vector.tensor_copy(sAB[0:64, :], poA[0:64, :])
                    nc.vector.tensor_copy(sAB[64:128, :], poB[0:64, :])

                def sums():
                    nc.vector.tensor_copy(s2[0:1, :], poA[64:65, :])
                    nc.vector.tensor_copy(s2[32:33, :], poB[64:65, :])
                    nc.vector.reciprocal_approx_fast(r2[:], s2[:])
                    if qt == 0:
                        nc.gpsimd.tensor_copy(r2b[:], r2[:])
                    else:
                        nc.vector.tensor_copy(r2b[:], r2[:])

                # leftover fills (dedup of positions) drain here; on the
                # last hp the rr chain is the critical path into the tail,
                # so its DVE ops go first
                if last_hp:
                    sums()
                    for tcn_et in fills:
                        outproj_group(*tcn_et)
                    evac()
                else:
                    for tcn_et in fills:
                        outproj_group(*tcn_et)
                    evac()
                    sums()

                def emit_rr(r2b=r2b, sAB=sAB, hp=hp, qs=qs):
                    rr = yop.tile([128, 512], fp, name="rr", tag="yo")
                    nc.tensor.matmul(rr[:], ind2b_r[:], r2b[:],
                                     start=True, stop=True,
                                     skip_group_check=True)
                    nc.vector.tensor_mul(oTu[:, hp, qs], sAB[:], rr[:])

                pending_rr[0] = emit_rr

            for idx in range(len(iters) + LA):
                if idx < len(iters):
                    emit_sea(idx)
                if idx >= LA:
                    emit_pv(idx - LA)
            # tail: output projection for the last token block (qt=3)
            flush_rr()
            for tcn in range(12, 16):
                for et in range(4):
                    outproj_group(tcn, et)


def _get_program():
    if "nc" in _CACHE:
        return _CACHE["nc"]
    import concourse.tile as tile
    from concourse import bacc, mybir

    nc = bacc.Bacc("TRN2", target_bir_lowering=False, debug=False,
                   num_devices=NCORES)
    fp = mybir.dt.float32
    bf = mybir.dt.bfloat16
    io = {
        "xT": nc.dram_tensor("xT", [4, 4, 128, 4, 512], bf,
                             kind="ExternalInput"),
        "wq": nc.dram_tensor("wq", [128, 16, 512], bf, kind="ExternalInput"),
        "wkv": nc.dram_tensor("wkv", [128, 16, 256], bf, kind="ExternalInput"),
        "wo": nc.dram_tensor("wo", [128, 4, 2048], bf, kind="ExternalInput"),
        "cosT": nc.dram_tensor("cosT", [128, 2048], bf, kind="ExternalInput"),
        "sinT": nc.dram_tensor("sinT", [128, 2048], bf, kind="ExternalInput"),
        "ident": nc.dram_tensor("ident", [128, 64], bf, kind="ExternalInput"),
        "ind2b": nc.dram_tensor("ind2b", [33, 128], bf, kind="ExternalInput"),
        "ones16": nc.dram_tensor("ones16", [128, 16], bf, kind="ExternalInput"),
        "y": nc.dram_tensor("y", [16, 4, 128, 512], bf, kind="ExternalOutput"),
    }
    with tile.TileContext(nc) as tc:
        _build_kernel(tc, nc, io, mybir)
    nc.compile()
    _CACHE["nc"] = nc
    return nc


def _run(inputs, trace=False):
    from concourse.bass_utils import run_bass_kernel_spmd

    nc = _get_program()
    in_maps = _host_prep(**inputs)
    res = run_bass_kernel_spmd(nc, in_maps, core_ids=list(range(NCORES)),
                               trace=trace)
    parts = [r_["y"].astype(np.float32).transpose(0, 2, 1, 3).reshape(SEQ, DIM)
             for r_ in res.results]
    out = np.stack([
        parts[0] + parts[1] + parts[2] + parts[3],
        parts[4] + parts[5] + parts[6] + parts[7],
    ]).astype(np.float32)
    return out, res


def kernel(**inputs):
    out, _ = _run(inputs, trace=False)
    return out

